# revision 5
# baseline (speedup 1.0000x reference)
"""Trainium2 Bass kernel for nn_DimeNetOutput (gnn message passing).

Computes, for E=1M edges / N=100K nodes / D=64:
    x        = (edge_attr @ We + be) * msg_emb          # [E, 64]
    node_emb = segment_sum(x, edge_dst, N)              # [N, 64]
    h        = relu(node_emb @ W1 + b1)  (applied 3x, same weights)
    out      = h @ W4                                   # [N, 64]

Strategy (8 NeuronCores, node-range sharding -> no collectives):
  * Host: sort edges by dst; core c owns nodes [c*12500, (c+1)*12500).
    Each node's edge list is 2-stacked (A/B partition halves, 64 feats each)
    into ceil(deg/2) pair-columns, which are split into equal L/R half-runs
    of q = ceil(cols/2) slots at the SAME offset in the L and R regions of
    the group (so one fp16 2x tensor_tensor adds them).  Groups of 1250
    nodes are padded to a fixed C = 7680 columns (L/R 3840 each).
  * Device per core, per group: PE matmul (We block-diag stationary) ->
    ACT psum->fp16 (+bias) -> DVE x *= msg (fp16 2x) -> DVE pair-add
    L+R -> DVE SEGMENTED scan (state = mask*state + t2, mask 0 at each
    node's first column) -> GPSIMD ap_gather of per-node end columns
    straight into node_t.  Then a 4-matmul MLP with relu+bias on ACT.
"""

import os

import numpy as np

# ----------------------------------------------------------------- constants
E = 1_000_000
N = 100_000
A_DIM = 16
D = 64
NCORES = 8

NPC = N // NCORES          # 12500 real nodes per core
G_NODES = 1250             # real nodes per group
NG = NPC // G_NODES        # 10 groups per core
CL = 3840                  # L (and R) columns per group
C = 2 * CL                 # 7680 padded columns per group
NCHUNK = C // 512          # 15 matmul chunks of 512 per group
CT = NG * C                # 76800 columns per core
ZERO_COL = CL              # index of the all-zero column in the scan window
GSLOT = 1280               # gather slots per group (1250 ends + pad to 16x)
GS16 = GSLOT // 16         # 80
NBANDS = (NCHUNK + 3) // 4  # 4 column bands in the packed attr layout
ACOLS = NBANDS * 512       # 2048 packed attr cols per group
NPC_DEV = NG * GSLOT       # 12800 device node slots per core
MLP_C1 = 256               # layer-1 chunk (50 chunks)
HCOLS = NPC_DEV // 2       # 6400 stacked columns for layers 2..4
MLP_C2 = 512               # layer-2..4 chunk

_F16 = np.float16
_PROG = {}


# ------------------------------------------------------------- host packing
def pack_inputs(edge_attr, msg_emb, edge_dst, We, be, W1, b1, W4):
    """Build the 8 per-core input maps (numpy only)."""
    dst = np.asarray(edge_dst).astype(np.int64).ravel()
    attr = np.asarray(edge_attr, dtype=np.float32)
    msg = np.asarray(msg_emb, dtype=np.float32)
    We = np.asarray(We, dtype=np.float32)
    be = np.asarray(be, dtype=np.float32).ravel()
    W1 = np.asarray(W1, dtype=np.float32)
    b1 = np.asarray(b1, dtype=np.float32).ravel()
    W4 = np.asarray(W4, dtype=np.float32)
    assert dst.shape == (E,) and attr.shape == (E, A_DIM) and msg.shape == (E, D)

    order = np.argsort(dst, kind="stable")
    deg = np.bincount(dst, minlength=N).astype(np.int64)
    estart = np.zeros(N + 1, np.int64)
    np.cumsum(deg, out=estart[1:])
    half = (deg + 1) // 2      # pair-columns per node
    qn = (half + 1) // 2       # L-run length per node

    # shared weight tensors; we2 replicated into all 4 32-row bands so each
    # band-matmul's stationary shares the rhs base partition (PE row tiling)
    we2 = np.zeros((128, 128), _F16)
    for u in range(4):
        we2[32 * u:32 * u + 16, 0:64] = We
        we2[32 * u + 16:32 * u + 32, 64:128] = We
    w1s = np.concatenate([W1, W1], axis=0).astype(np.float32)    # [128, 64]
    w1b = np.zeros((128, 128), _F16)
    w1b[0:64, 0:64] = W1
    w1b[64:128, 64:128] = W1
    w4b = np.zeros((128, 128), _F16)
    w4b[0:64, 0:64] = W4
    w4b[64:128, 64:128] = W4
    b1h = b1.reshape(64, 1).astype(np.float32)
    b1s = np.concatenate([b1, b1]).reshape(128, 1).astype(np.float32)
    bes = np.concatenate([be, be]).reshape(128, 1).astype(np.float32)

    in_maps = []
    for c in range(NCORES):
        nlo = c * NPC
        deg_c = deg[nlo:nlo + NPC]
        half_c = half[nlo:nlo + NPC]
        qn_c = qn[nlo:nlo + NPC]
        estart_c = estart[nlo:nlo + NPC]

        qn_g = qn_c.reshape(NG, G_NODES)
        qtot = qn_g.sum(axis=1)
        if not (qtot <= CL).all():
            raise RuntimeError(f"group overflow: max L cols {qtot.max()} > {CL}")
        cum_q = np.cumsum(qn_g, axis=1)
        colstart_g = cum_q - qn_g                    # within-group L col start
        endcol_g = cum_q - 1                         # node end col in L (-1 ok)

        # per pair-column arrays (real columns only)
        ncols = int(half_c.sum())
        node_of_col = np.repeat(np.arange(NPC), half_c)
        cstart_node = np.cumsum(half_c) - half_c
        k_arr = np.arange(ncols) - np.repeat(cstart_node, half_c)
        g_of_col = node_of_col // G_NODES
        nl = node_of_col % G_NODES                   # node idx within group
        q_col = qn_c[node_of_col]
        in_l = k_arr < q_col
        # L half at colstart + k; R half at CL + colstart + (k - q)
        within = np.where(in_l, k_arr, CL + k_arr - q_col)
        colpos = (g_of_col * C + colstart_g[g_of_col, nl] + within).astype(np.int64)
        es = estart_c[node_of_col]
        pA = es + k_arr
        pB = es + half_c[node_of_col] + k_arr
        validB = pB < es + deg_c[node_of_col]
        eA = order[pA]
        eB = order[np.minimum(pB, E - 1)]

        # msgT packed [128, CT] fp16: rows 0:64 = A-half features, 64:128 = B
        msgT = np.zeros((128, CT), _F16)
        msgT[0:64, colpos] = msg[eA].T
        msgT[64:128, colpos[validB]] = msg[eB[validB]].T

        # attr 2-block [32, CT] then band-packed [128, NG*ACOLS] fp16
        a2 = np.zeros((32, CT), np.float32)
        a2[0:16, colpos] = attr[eA].T
        a2[16:32, colpos[validB]] = attr[eB[validB]].T
        ap_ = np.zeros((128, NG * ACOLS), _F16)
        for g in range(NG):
            blk = a2[:, g * C:(g + 1) * C].reshape(32, NCHUNK, 512)
            for q in range(NCHUNK):
                u, b = q % 4, q // 4
                ap_[32 * u:32 * u + 32,
                    g * ACOLS + 512 * b:g * ACOLS + 512 * (b + 1)] = blk[:, q]

        # gather indices: per group slots = [prev_end(=-1->ZERO)] + endcols
        gidx = np.zeros((128, NG * GS16), np.int16)
        for g in range(NG):
            slots = np.full(GSLOT, ZERO_COL, np.int64)
            ends = endcol_g[g]
            slots[1:1 + G_NODES] = np.where(ends < 0, ZERO_COL, ends)
            wrapped = slots.reshape(GS16, 16).T      # [16, GS16]; i = s*16+p
            gidx[:, g * GS16:(g + 1) * GS16] = np.tile(wrapped, (8, 1))

        in_maps.append({
            "attrp": ap_, "msgT": msgT, "we2": we2,
            "w1s": w1s, "w1b": w1b, "w4b": w4b, "b1h": b1h, "b1s": b1s,
            "bes": bes, "gidx": gidx,
        })
    return in_maps


def unpack_output(results):
    """results: list of 8 dicts with 'outp' [128, HCOLS] f32 -> [N, 64]."""
    full = np.empty((N, D), np.float32)
    # device node m: beta=(m//MLP_C1)%2, blk=m//(2*MLP_C1), i=m%MLP_C1
    m = np.arange(NPC_DEV)
    beta, blk, i = (m // MLP_C1) % 2, m // (2 * MLP_C1), m % MLP_C1
    g, s = m // GSLOT, m % GSLOT
    valid = s < G_NODES
    node_of_m = (g * G_NODES + s)[valid]
    for c, r in enumerate(results):
        op_ = np.asarray(r["outp"], np.float32)       # [128, HCOLS]
        o = op_.reshape(2, 64, HCOLS // MLP_C1, MLP_C1)  # [beta, d, blk, i]
        vals = o[beta[valid], :, blk[valid], i[valid]]   # ordered by node
        full[c * NPC:(c + 1) * NPC] = vals[np.argsort(node_of_m)]
    return full


# ---------------------------------------------------------- device program
DEBUG_OUTS = os.environ.get("GNN_DEBUG_OUTS", "0") == "1"


def build_device_program(tc, outs, ins, n_reps=1):
    import concourse.mybir as mybir

    nc = tc.nc
    f16 = mybir.dt.float16
    f32 = mybir.dt.float32
    Alu = mybir.AluOpType
    Act = mybir.ActivationFunctionType

    outp = outs["outp"]

    with tc.sbuf_pool(name="cpool", bufs=1) as cp:
        we2_t = cp.tile_from(ins["we2"])
        w1s_t = cp.tile_from(ins["w1s"])
        w1b_t = cp.tile_from(ins["w1b"])
        w4b_t = cp.tile_from(ins["w4b"])
        b1h_t = cp.tile_from(ins["b1h"])
        b1s_t = cp.tile_from(ins["b1s"])
        bes_t = cp.tile_from(ins["bes"])
        gidx_t = cp.tile_from(ins["gidx"])
        node_t = cp.tile([128, NPC_DEV], f32)
        # warmup ap_gather: absorbs the one-time GPSIMD ucode IRAM load
        # (~60us) before the pipeline needs real gathers
        zidx_t = cp.tile([128, 1], mybir.dt.int16)
        nc.vector.memset(zidx_t[:], 0)
        wdum_t = cp.tile([128, 16], f32)
        nc.gpsimd.ap_gather(wdum_t[:], b1s_t[:], zidx_t[:, 0:1],
                            channels=128, num_elems=1, d=1, num_idxs=16)

        h1_t = cp.tile([128, HCOLS], f16)
        zcol_t = cp.tile([128, 1], f16)
        nc.vector.memset(zcol_t[:], 0.0)
        nc.gpsimd.memset(node_t[:], 0.0)
        h2_t = cp.tile([128, HCOLS], f16)
        for _rep in range(n_reps):
            _one_pass(tc, nc, outs, ins, cp, mybir,
                      we2_t, w1s_t, w1b_t, w4b_t, b1h_t, b1s_t, bes_t,
                      gidx_t, node_t, h1_t, h2_t, zcol_t)


def _one_pass(tc, nc, outs, ins, cp, mybir,
              we2_t, w1s_t, w1b_t, w4b_t, b1h_t, b1s_t, bes_t,
              gidx_t, node_t, h1_t, h2_t, zcol_t):
        f16 = mybir.dt.float16
        f32 = mybir.dt.float32
        Alu = mybir.AluOpType
        Act = mybir.ActivationFunctionType
        outp = outs["outp"]
        with tc.sbuf_pool(name="wpool", bufs=2) as wp, \
             tc.sbuf_pool(name="wq", bufs=1) as wq, \
             tc.sbuf_pool(name="wv", bufs=2) as wv, \
             tc.sbuf_pool(name="opool", bufs=2) as obp, \
             tc.tile_pool(name="pspool", bufs=2, space="PSUM") as pp, \
             tc.tile_pool(name="l1pool", bufs=2, space="PSUM") as lp, \
             tc.tile_pool(name="mpspool", bufs=2, space="PSUM") as mpp:
            l1_done = 0
            l2_done = 0
            for g in range(NG):
                msg_t = wp.tile([128, C], f16, tag="msg")
                nc.sync.dma_start(msg_t[:], ins["msgT"][:, g * C:(g + 1) * C])
                attr_t = wp.tile([128, ACOLS], f16, tag="attr")
                nc.sync.dma_start(attr_t[:],
                                  ins["attrp"][:, g * ACOLS:(g + 1) * ACOLS])
                x_t = wp.tile([128, C], f16, tag="x")
                t2_t = wq.tile([128, CL], f16, tag="t2")
                win_t = wv.tile([128, CL + 1], f32, tag="win")

                # 2-chunk PSUM bands (4 banks for stage A, leaving room for
                # the interleaved layer-1 pool)
                for t in range((NCHUNK + 1) // 2):
                    qlo, qhi = 2 * t, min(2 * t + 2, NCHUNK)
                    width = (qhi - qlo) * 512
                    ps_t = pp.tile([128, 1024], f32, tag="ps")
                    for j in range(qhi - qlo):
                        q = qlo + j
                        u, b = q % 4, q // 4
                        nc.tensor.matmul(
                            ps_t[:, 512 * j:512 * (j + 1)],
                            we2_t[32 * u:32 * (u + 1), :],
                            attr_t[32 * u:32 * (u + 1), 512 * b:512 * (b + 1)],
                            start=True, stop=True,
                            tile_position=(32 * u, 0))
                    # ACT: fp16 copy of xlin + bias (keeps DVE mul in 2x mode)
                    nc.scalar.activation(
                        x_t[:, 1024 * t:1024 * t + width],
                        ps_t[:, :width],
                        Act.Identity, bias=bes_t[:, 0:1])
                # DVE: x *= msg (all-fp16 SBUF, 2x_1P)
                nc.vector.tensor_tensor(
                    x_t[:], x_t[:], msg_t[:], op=Alu.mult)
                # DVE: pair-add of the L and R half-runs (fp16 2x)
                nc.vector.tensor_tensor(
                    t2_t[:], x_t[:, 0:CL], x_t[:, CL:C], op=Alu.add)
                # DVE: running f32 prefix sum along the L columns
                nc.vector.tensor_tensor_scan(
                    win_t[:, 0:CL],
                    zcol_t[:].to_broadcast([128, CL]),
                    t2_t[:],
                    0.0, op0=Alu.add, op1=Alu.add)
                nc.vector.memset(win_t[:, CL:CL + 1], 0.0)

                # GPSIMD: gather prefix values at [prev_end; end] columns,
                # then diff into exact per-node sums (keeps DVE free)
                s1_t = wq.tile([128, GSLOT], f32, tag="s1")
                nc.gpsimd.ap_gather(
                    s1_t[:], win_t[:],
                    gidx_t[:, g * GS16:(g + 1) * GS16],
                    channels=128, num_elems=CL + 1, d=1, num_idxs=GSLOT)
                nc.gpsimd.tensor_tensor(
                    node_t[:, g * GSLOT:g * GSLOT + G_NODES],
                    s1_t[:, 1:1 + G_NODES],
                    s1_t[:, 0:G_NODES],
                    op=Alu.subtract)

                # MLP layer 1 for this group's nodes, interleaved: two
                # col-tiled matmuls fill a [128, 256] PSUM tile, one ACT
                l1_hi = (g + 1) * GSLOT // 512
                for b2 in range(l1_done, l1_hi):
                    pt = lp.tile([128, 256], f32, tag="l1")
                    nc.tensor.matmul(pt[0:64, :], w1s_t[:],
                                     node_t[:, 512 * b2:512 * b2 + 256],
                                     start=True, stop=True)
                    nc.tensor.matmul(pt[64:128, :], w1s_t[:],
                                     node_t[:, 512 * b2 + 256:512 * b2 + 512],
                                     start=True, stop=True,
                                     tile_position=(0, 64))
                    nc.scalar.activation(
                        h1_t[:, MLP_C1 * b2:MLP_C1 * (b2 + 1)],
                        pt[:], Act.Relu, bias=b1s_t[:, 0:1])
                l1_done = l1_hi
                l2_hi = (l1_done * MLP_C1) // MLP_C2
                for cc in range(l2_done, l2_hi):
                    _mlp_tail_chunk(nc, mpp, obp, outp, h1_t, h2_t,
                                    w1b_t, w4b_t, b1s_t, cc * MLP_C2,
                                    min(MLP_C2, HCOLS - cc * MLP_C2))
                l2_done = l2_hi
                if DEBUG_OUTS and g == 0:
                    nc.sync.dma_start(outs["dbg_x"][:], x_t[:])
                    nc.sync.dma_start(outs["dbg_win"][:], win_t[:])
            if DEBUG_OUTS:
                nc.sync.dma_start(outs["dbg_node"][:], node_t[:])

        # ---------------------------------------------- MLP tail flush
            for c0 in range(l2_done * MLP_C2, HCOLS, MLP_C2):
                _mlp_tail_chunk(nc, mpp, obp, outp, h1_t, h2_t,
                                w1b_t, w4b_t, b1s_t, c0,
                                min(MLP_C2, HCOLS - c0))


def _mlp_tail_chunk(nc, mpp, obp, outp, h1_t, h2_t, w1b_t, w4b_t, b1s_t,
                    c0, w):
    """Layers 2-4 + output DMA for h-cols [c0, c0+w); h3 aliases h1."""
    import concourse.mybir as mybir
    f32 = mybir.dt.float32
    Act = mybir.ActivationFunctionType
    pt = mpp.tile([128, 512], f32, tag="mp", name="pt")
    nc.tensor.matmul(pt[:, 0:w], w1b_t[:], h1_t[:, c0:c0 + w],
                     start=True, stop=True)
    nc.scalar.activation(h2_t[:, c0:c0 + w], pt[:, 0:w],
                         Act.Relu, bias=b1s_t[:, 0:1])
    pt = mpp.tile([128, 512], f32, tag="mp", name="pt")
    nc.tensor.matmul(pt[:, 0:w], w1b_t[:], h2_t[:, c0:c0 + w],
                     start=True, stop=True)
    nc.scalar.activation(h1_t[:, c0:c0 + w], pt[:, 0:w],
                         Act.Relu, bias=b1s_t[:, 0:1])
    pt = mpp.tile([128, 512], f32, tag="mp", name="pt")
    nc.tensor.matmul(pt[:, 0:w], w4b_t[:], h1_t[:, c0:c0 + w],
                     start=True, stop=True)
    ob = obp.tile([128, MLP_C2], f32, tag="ob", name="ob")
    nc.scalar.copy(ob[:, 0:w], pt[:, 0:w])
    nc.sync.dma_start(outp[:, c0:c0 + w], ob[:, 0:w])


def build_program(n_reps=1):
    """Build (once per n_reps) the Bacc program + dram tensor APs."""
    if ("nc", n_reps) in _PROG:
        return _PROG[("nc", n_reps)]
    import concourse.bacc as bacc
    import concourse.mybir as mybir
    import concourse.tile as tile

    nc = bacc.Bacc("TRN2", debug=False, enable_asserts=False)
    f16, f32, i16 = mybir.dt.float16, mybir.dt.float32, mybir.dt.int16
    ins = {
        "attrp": nc.dram_tensor("attrp", [128, NG * ACOLS], f16,
                                kind="ExternalInput").ap(),
        "msgT": nc.dram_tensor("msgT", [128, CT], f16,
                               kind="ExternalInput").ap(),
        "we2": nc.dram_tensor("we2", [128, 128], f16, kind="ExternalInput").ap(),
        "w1s": nc.dram_tensor("w1s", [128, 64], f32, kind="ExternalInput").ap(),
        "w1b": nc.dram_tensor("w1b", [128, 128], f16, kind="ExternalInput").ap(),
        "w4b": nc.dram_tensor("w4b", [128, 128], f16, kind="ExternalInput").ap(),
        "b1h": nc.dram_tensor("b1h", [64, 1], f32, kind="ExternalInput").ap(),
        "b1s": nc.dram_tensor("b1s", [128, 1], f32, kind="ExternalInput").ap(),
        "bes": nc.dram_tensor("bes", [128, 1], f32, kind="ExternalInput").ap(),
        "gidx": nc.dram_tensor("gidx", [128, NG * GS16], i16,
                               kind="ExternalInput").ap(),
    }
    outs = {
        "outp": nc.dram_tensor("outp", [128, HCOLS], f32,
                               kind="ExternalOutput").ap(),
    }
    if DEBUG_OUTS:
        outs["dbg_x"] = nc.dram_tensor("dbg_x", [128, C], f16,
                                       kind="ExternalOutput").ap()
        outs["dbg_win"] = nc.dram_tensor("dbg_win", [128, CL + 1], f32,
                                         kind="ExternalOutput").ap()
        outs["dbg_node"] = nc.dram_tensor("dbg_node", [128, NPC_DEV], f32,
                                          kind="ExternalOutput").ap()
    with tile.TileContext(nc) as tc:
        build_device_program(tc, outs, ins, n_reps=n_reps)
    nc.compile()
    _PROG[("nc", n_reps)] = nc
    return nc


# ------------------------------------------------------------------ kernel
def kernel(edge_attr, msg_emb, edge_dst, num_nodes, We, be, W1, b1, W4,
           **_unused):
    assert int(num_nodes) == N
    in_maps = pack_inputs(edge_attr, msg_emb, edge_dst, We, be, W1, b1, W4)
    nc = build_program()

    from concourse.bass_utils import run_bass_kernel_spmd
    trace = os.environ.get("GNN_TRACE", "0") == "1"
    res = run_bass_kernel_spmd(nc, in_maps, core_ids=list(range(NCORES)),
                               trace=trace)
    kernel.last_results = res
    return unpack_output(res.results)



# revision 18
# speedup vs baseline: 3.2824x; 3.2824x over previous
"""Trainium2 Bass kernel for nn_DimeNetOutput (gnn message passing).

Computes, for E=1M edges / N=100K nodes / D=64:
    x        = (edge_attr @ We + be) * msg_emb          # [E, 64]
    node_emb = segment_sum(x, edge_dst, N)              # [N, 64]
    h        = relu(node_emb @ W1 + b1)  (applied 3x, same weights)
    out      = h @ W4                                   # [N, 64]

Strategy (8 NeuronCores, node-range sharding -> no collectives):
  * Host: sort edges by dst; core c owns nodes [c*12500, (c+1)*12500).
    Each node's edge list is 2-stacked (A/B partition halves, 64 feats
    each) into half=ceil(deg/2) pair-columns.  Nodes are binned by
    q = ceil(half/2) into fixed-capacity width-q buckets (template
    B_q = [40,390,574,238,32,4,2] per group of 1280 node slots, 10
    groups per core; a node may spill into a wider bucket).  A node in
    a width-q bucket owns q consecutive columns in the group's L
    region and q in the R region (same offset); its first min(half,q)
    pair-columns go to L, the rest to R.  Unused columns are zero.
  * Device per core, per group: PE matmul (We block-diag stationary) ->
    ACT psum->fp16 (+bias) -> DVE x *= msg (fp16 2x) -> DVE pair-add
    L+R into a 5-group t2 batch buffer.  Every 5 groups, per-bucket
    STRIDED reductions (3D APs [128, 5, B_q], stride q along columns)
    sum each node's q columns straight into node_t -- no gather, no
    GPSIMD.  Then a 4-matmul MLP with relu+bias on ACT.
"""

import os

import numpy as np

# ----------------------------------------------------------------- constants
E = 1_000_000
N = 100_000
A_DIM = 16
D = 64
NCORES = 8

NPC = N // NCORES          # 12500 real nodes per core
NG = 10                    # groups per core
CL = 3840                  # L (and R) columns per group
C = 2 * CL                 # 7680 padded columns per group
NCHUNK = C // 512          # 15 matmul chunks of 512 per group
CT = NG * C                # 76800 columns per core
SLOTG = 1280               # node slots per group
NBANDS = (NCHUNK + 3) // 4  # 4 column bands in the packed attr layout
ACOLS = NBANDS * 512       # 2048 packed attr cols per group
NPC_DEV = NG * SLOTG       # 12800 device node slots per core
MLP_C1 = 256               # layer-1 chunk (50 chunks)
HCOLS = NPC_DEV // 2       # 6400 stacked columns for layers 2..4
MLP_C2 = 512               # layer-2..4 chunk
TB = 5                     # groups per strided-reduction batch

QMAX = 7
BQ = [0, 40, 390, 574, 238, 32, 4, 2]          # bucket capacity per q
OFFL = [0] * (QMAX + 2)                         # col offset of bucket q in L
SOFF = [0] * (QMAX + 2)                         # slot offset of bucket q
for _q in range(1, QMAX + 1):
    OFFL[_q + 1] = OFFL[_q] + BQ[_q] * _q
    SOFF[_q + 1] = SOFF[_q] + BQ[_q]
assert OFFL[QMAX + 1] <= CL and SOFF[QMAX + 1] == SLOTG

_F16 = np.float16
_PROG = {}
ABLATE = set()      # dev-only: stage names to omit from the device program


# ------------------------------------------------------------- host packing
def pack_inputs(edge_attr, msg_emb, edge_dst, We, be, W1, b1, W4):
    """Build the 8 per-core input maps + slot->node maps (numpy only)."""
    dst = np.asarray(edge_dst).astype(np.int64).ravel()
    attr = np.asarray(edge_attr, dtype=np.float32)
    msg = np.asarray(msg_emb, dtype=np.float32)
    We = np.asarray(We, dtype=np.float32)
    be = np.asarray(be, dtype=np.float32).ravel()
    W1 = np.asarray(W1, dtype=np.float32)
    b1 = np.asarray(b1, dtype=np.float32).ravel()
    W4 = np.asarray(W4, dtype=np.float32)
    assert dst.shape == (E,) and attr.shape == (E, A_DIM) and msg.shape == (E, D)

    order = np.argsort(dst, kind="stable")
    deg = np.bincount(dst, minlength=N).astype(np.int64)
    estart = np.zeros(N + 1, np.int64)
    np.cumsum(deg, out=estart[1:])

    # shared weight tensors; we2 replicated into all 4 32-row bands so each
    # band-matmul's stationary shares the rhs base partition (PE row tiling)
    we2 = np.zeros((128, 128), _F16)
    for u in range(4):
        we2[32 * u:32 * u + 16, 0:64] = We
        we2[32 * u + 16:32 * u + 32, 64:128] = We
    w1s = np.concatenate([W1, W1], axis=0).astype(np.float32)    # [128, 64]
    w1b = np.zeros((128, 128), _F16)
    w1b[0:64, 0:64] = W1
    w1b[64:128, 64:128] = W1
    w4b = np.zeros((128, 128), _F16)
    w4b[0:64, 0:64] = W4
    w4b[64:128, 64:128] = W4
    b1s = np.concatenate([b1, b1]).reshape(128, 1).astype(np.float32)
    bes = np.concatenate([be, be]).reshape(128, 1).astype(np.float32)

    in_maps, slotmaps = [], []
    for c in range(NCORES):
        nlo = c * NPC
        deg_c = deg[nlo:nlo + NPC]
        estart_c = estart[nlo:nlo + NPC]
        half_c = (deg_c + 1) // 2
        qn = np.maximum((half_c + 1) // 2, 1)

        # --- assign nodes to (group, bucket, slot); spill upward if full
        used = np.zeros((NG, QMAX + 1), np.int64)
        node_g = np.empty(NPC, np.int64)
        node_qb = np.empty(NPC, np.int64)
        node_s = np.empty(NPC, np.int64)
        pending = np.array([], dtype=np.int64)
        for q in range(QMAX, 0, -1):
            cand = np.concatenate([np.where(qn == q)[0], pending])
            pending = np.array([], dtype=np.int64)
            pos = 0
            for qp in range(q, QMAX + 1):
                if pos >= len(cand):
                    break
                for g in range(NG):
                    freeg = BQ[qp] - used[g, qp]
                    if freeg <= 0 or pos >= len(cand):
                        continue
                    take = min(freeg, len(cand) - pos)
                    sel = cand[pos:pos + take]
                    node_g[sel] = g
                    node_qb[sel] = qp
                    node_s[sel] = used[g, qp] + np.arange(take)
                    used[g, qp] += take
                    pos += take
            if pos < len(cand):
                raise RuntimeError(
                    f"core {c}: bucket overflow at q={q} "
                    f"({len(cand) - pos} nodes unplaced)")

        # --- per pair-column arrays (real columns only)
        ncols = int(half_c.sum())
        node_of_col = np.repeat(np.arange(NPC), half_c)
        cstart_node = np.cumsum(half_c) - half_c
        k_arr = np.arange(ncols) - np.repeat(cstart_node, half_c)
        qb_col = node_qb[node_of_col]
        ln_col = np.minimum(half_c, node_qb)[node_of_col]
        base_l = (node_g * C + np.array(OFFL)[node_qb] +
                  node_s * node_qb)[node_of_col]
        in_l = k_arr < ln_col
        colpos = np.where(in_l, base_l + k_arr,
                          CL + base_l + (k_arr - ln_col)).astype(np.int64)
        es = estart_c[node_of_col]
        pA = es + k_arr
        pB = es + half_c[node_of_col] + k_arr
        validB = pB < es + deg_c[node_of_col]
        eA = order[pA]
        eB = order[np.minimum(pB, E - 1)]

        # msgT packed [128, CT] fp16: rows 0:64 = A-half features, 64:128 = B
        msgT = np.zeros((128, CT), _F16)
        msgT[0:64, colpos] = msg[eA].T
        msgT[64:128, colpos[validB]] = msg[eB[validB]].T

        # attr 2-block [32, CT] then band-packed [128, NG*ACOLS] fp16
        a2 = np.zeros((32, CT), np.float32)
        a2[0:16, colpos] = attr[eA].T
        a2[16:32, colpos[validB]] = attr[eB[validB]].T
        ap_ = np.zeros((128, NG * ACOLS), _F16)
        for g in range(NG):
            blk = a2[:, g * C:(g + 1) * C].reshape(32, NCHUNK, 512)
            for q in range(NCHUNK):
                u, b = q % 4, q // 4
                ap_[32 * u:32 * u + 32,
                    g * ACOLS + 512 * b:g * ACOLS + 512 * (b + 1)] = blk[:, q]

        # slot -> node map (-1 = padding slot)
        slot_node = np.full(NPC_DEV, -1, np.int64)
        m = node_g * SLOTG + np.array(SOFF)[node_qb] + node_s
        slot_node[m] = np.arange(NPC)

        in_maps.append({
            "attrp": ap_, "msgT": msgT, "we2": we2,
            "w1s": w1s, "w1b": w1b, "w4b": w4b, "b1s": b1s, "bes": bes,
        })
        slotmaps.append(slot_node)
    return in_maps, slotmaps


def unpack_output(results, slotmaps):
    """results: 8 dicts with 'outp' [128, HCOLS] f32 -> [N, 64]."""
    full = np.empty((N, D), np.float32)
    # device node m: beta=(m//MLP_C1)%2, blk=m//(2*MLP_C1), i=m%MLP_C1
    m = np.arange(NPC_DEV)
    beta, blk, i = (m // MLP_C1) % 2, m // (2 * MLP_C1), m % MLP_C1
    for c, r in enumerate(results):
        slot_node = slotmaps[c]
        valid = slot_node >= 0
        op_ = np.asarray(r["outp"], np.float32)       # [128, HCOLS]
        o = op_.reshape(2, 64, HCOLS // MLP_C1, MLP_C1)  # [beta, d, blk, i]
        vals = o[beta[valid], :, blk[valid], i[valid]]   # per valid slot
        full[c * NPC + slot_node[valid]] = vals
    return full


# ---------------------------------------------------------- device program
def build_device_program(tc, outs, ins, n_reps=1):
    import concourse.mybir as mybir

    nc = tc.nc
    f16 = mybir.dt.float16
    f32 = mybir.dt.float32

    with tc.sbuf_pool(name="cpool", bufs=1) as cp:
        we2_t = cp.tile_from(ins["we2"])
        w1s_t = cp.tile_from(ins["w1s"])
        w1b_t = cp.tile_from(ins["w1b"])
        w4b_t = cp.tile_from(ins["w4b"])
        b1s_t = cp.tile_from(ins["b1s"])
        bes_t = cp.tile_from(ins["bes"])
        node_t = cp.tile([128, NPC_DEV], f32)
        t2b_t = cp.tile([128, TB * CL], f16)
        if "mlp" not in ABLATE:
            h1_t = cp.tile([128, HCOLS], f16)
            h2_t = cp.tile([128, HCOLS], f16)
        else:
            h1_t = h2_t = None
        for _rep in range(n_reps):
            _one_pass(tc, nc, outs, ins, mybir,
                      we2_t, w1s_t, w1b_t, w4b_t, b1s_t, bes_t,
                      node_t, t2b_t, h1_t, h2_t)


def _emit_batch_reduce(nc, Alu, node_t, t2b_t, g0):
    """Per-bucket strided sums of the TB groups ending at g0+TB-1."""
    t3 = t2b_t[:].rearrange("p (G c) -> p G c", G=TB)    # [128, TB, CL]
    n3 = node_t[:, g0 * SLOTG:(g0 + TB) * SLOTG].rearrange(
        "p (G s) -> p G s", G=TB)                        # [128, TB, SLOTG]
    for q in range(1, QMAX + 1):
        o, B = OFFL[q], BQ[q]
        nv = n3[:, :, SOFF[q]:SOFF[q] + B]
        if q == 1:
            nc.scalar.copy(nv, t3[:, :, o:o + B])
            continue
        nc.vector.tensor_tensor(nv, t3[:, :, o:o + B * q:q],
                                t3[:, :, o + 1:o + B * q:q], op=Alu.add)
        for j in range(2, q):
            nc.vector.tensor_tensor(nv, nv, t3[:, :, o + j:o + B * q:q],
                                    op=Alu.add)


def _one_pass(tc, nc, outs, ins, mybir,
              we2_t, w1s_t, w1b_t, w4b_t, b1s_t, bes_t,
              node_t, t2b_t, h1_t, h2_t):
    f16 = mybir.dt.float16
    f32 = mybir.dt.float32
    Alu = mybir.AluOpType
    Act = mybir.ActivationFunctionType
    outp = outs["outp"]
    with tc.sbuf_pool(name="wpool", bufs=2) as wp, \
         tc.tile_pool(name="pspool", bufs=2, space="PSUM") as pp, \
         tc.tile_pool(name="l1pool", bufs=2, space="PSUM") as lp, \
         tc.sbuf_pool(name="opool", bufs=2) as obp, \
         tc.tile_pool(name="mpspool", bufs=2, space="PSUM") as mpp:
        l1_done = 0
        l2_done = 0
        for g in range(NG):
            if "dma" not in ABLATE:
                msg_t = wp.tile([128, C], f16, tag="msg")
                attr_t = wp.tile([128, ACOLS], f16, tag="attr")
                nc.sync.dma_start(msg_t[:],
                                  ins["msgT"][:, g * C:(g + 1) * C])
                nc.sync.dma_start(attr_t[:],
                                  ins["attrp"][:, g * ACOLS:(g + 1) * ACOLS])
            x_t = wp.tile([128, C], f16, tag="x")

            # 2-chunk PSUM bands for stage A
            for t in range(0 if "mmA" in ABLATE else (NCHUNK + 1) // 2):
                qlo, qhi = 2 * t, min(2 * t + 2, NCHUNK)
                width = (qhi - qlo) * 512
                ps_t = pp.tile([128, 1024], f32, tag="ps")
                for j in range(qhi - qlo):
                    q = qlo + j
                    u, b = q % 4, q // 4
                    nc.tensor.matmul(
                        ps_t[:, 512 * j:512 * (j + 1)],
                        we2_t[32 * u:32 * (u + 1), :],
                        attr_t[32 * u:32 * (u + 1), 512 * b:512 * (b + 1)],
                        start=True, stop=True,
                        tile_position=(32 * u, 0))
                # ACT: fp16 copy of xlin + bias (keeps DVE mul in 2x mode)
                nc.scalar.activation(
                    x_t[:, 1024 * t:1024 * t + width],
                    ps_t[:, :width],
                    Act.Identity, bias=bes_t[:, 0:1])
            if "dve" not in ABLATE:
                # DVE: x *= msg (all-fp16 SBUF, 2x_1P)
                nc.vector.tensor_tensor(
                    x_t[:], x_t[:], msg_t[:], op=Alu.mult)
                # DVE: pair-add of L and R halves into the batch buffer
                nc.vector.tensor_tensor(
                    t2b_t[:, (g % TB) * CL:(g % TB + 1) * CL],
                    x_t[:, 0:CL], x_t[:, CL:C], op=Alu.add)

            if g % TB == TB - 1:
                if "reduce" not in ABLATE:
                    _emit_batch_reduce(nc, Alu, node_t, t2b_t, g - TB + 1)
                # MLP layer 1 for the finished groups' nodes: two
                # col-tiled matmuls fill a [128, 256] PSUM tile, one ACT
                l1_hi = (0 if "mlp" in ABLATE
                         else ((g + 1) * SLOTG) // 512)
                for b2 in range(l1_done, l1_hi):
                    pt = lp.tile([128, 256], f32, tag="l1")
                    nc.tensor.matmul(pt[0:64, :], w1s_t[:],
                                     node_t[:, 512 * b2:512 * b2 + 256],
                                     start=True, stop=True)
                    nc.tensor.matmul(pt[64:128, :], w1s_t[:],
                                     node_t[:, 512 * b2 + 256:512 * b2 + 512],
                                     start=True, stop=True,
                                     tile_position=(0, 64))
                    nc.scalar.activation(
                        h1_t[:, MLP_C1 * b2:MLP_C1 * (b2 + 1)],
                        pt[:], Act.Relu, bias=b1s_t[:, 0:1])
                l1_done = l1_hi
                l2_hi = (l1_done * MLP_C1) // MLP_C2
                for cc in range(l2_done, l2_hi):
                    _mlp_tail_chunk(nc, mpp, obp, outp, h1_t, h2_t,
                                    w1b_t, w4b_t, b1s_t, cc * MLP_C2,
                                    min(MLP_C2, HCOLS - cc * MLP_C2))
                l2_done = l2_hi

        # ---------------------------------------------- MLP tail flush
        if "mlp" not in ABLATE:
            for c0 in range(l2_done * MLP_C2, HCOLS, MLP_C2):
                _mlp_tail_chunk(nc, mpp, obp, outp, h1_t, h2_t,
                                w1b_t, w4b_t, b1s_t, c0,
                                min(MLP_C2, HCOLS - c0))


def _mlp_tail_chunk(nc, mpp, obp, outp, h1_t, h2_t, w1b_t, w4b_t, b1s_t,
                    c0, w):
    """Layers 2-4 + output DMA for h-cols [c0, c0+w); h3 aliases h1."""
    import concourse.mybir as mybir
    f32 = mybir.dt.float32
    Act = mybir.ActivationFunctionType
    pt = mpp.tile([128, 512], f32, tag="mp", name="pt")
    nc.tensor.matmul(pt[:, 0:w], w1b_t[:], h1_t[:, c0:c0 + w],
                     start=True, stop=True)
    nc.scalar.activation(h2_t[:, c0:c0 + w], pt[:, 0:w],
                         Act.Relu, bias=b1s_t[:, 0:1])
    pt = mpp.tile([128, 512], f32, tag="mp", name="pt")
    nc.tensor.matmul(pt[:, 0:w], w1b_t[:], h2_t[:, c0:c0 + w],
                     start=True, stop=True)
    nc.scalar.activation(h1_t[:, c0:c0 + w], pt[:, 0:w],
                         Act.Relu, bias=b1s_t[:, 0:1])
    pt = mpp.tile([128, 512], f32, tag="mp", name="pt")
    nc.tensor.matmul(pt[:, 0:w], w4b_t[:], h1_t[:, c0:c0 + w],
                     start=True, stop=True)
    ob = obp.tile([128, MLP_C2], f32, tag="ob", name="ob")
    nc.scalar.copy(ob[:, 0:w], pt[:, 0:w])
    nc.sync.dma_start(outp[:, c0:c0 + w], ob[:, 0:w])


def build_program(n_reps=1):
    """Build (once per n_reps/ablation) the Bacc program + dram APs."""
    key = ("nc", n_reps, tuple(sorted(ABLATE)))
    if key in _PROG:
        return _PROG[key]
    import concourse.bacc as bacc
    import concourse.mybir as mybir
    import concourse.tile as tile

    nc = bacc.Bacc("TRN2", debug=False, enable_asserts=False)
    f16, f32 = mybir.dt.float16, mybir.dt.float32
    ins = {
        "attrp": nc.dram_tensor("attrp", [128, NG * ACOLS], f16,
                                kind="ExternalInput").ap(),
        "msgT": nc.dram_tensor("msgT", [128, CT], f16,
                               kind="ExternalInput").ap(),
        "we2": nc.dram_tensor("we2", [128, 128], f16, kind="ExternalInput").ap(),
        "w1s": nc.dram_tensor("w1s", [128, 64], f32, kind="ExternalInput").ap(),
        "w1b": nc.dram_tensor("w1b", [128, 128], f16, kind="ExternalInput").ap(),
        "w4b": nc.dram_tensor("w4b", [128, 128], f16, kind="ExternalInput").ap(),
        "b1s": nc.dram_tensor("b1s", [128, 1], f32, kind="ExternalInput").ap(),
        "bes": nc.dram_tensor("bes", [128, 1], f32, kind="ExternalInput").ap(),
    }
    outs = {
        "outp": nc.dram_tensor("outp", [128, HCOLS], f32,
                               kind="ExternalOutput").ap(),
    }
    with tile.TileContext(nc) as tc:
        build_device_program(tc, outs, ins, n_reps=n_reps)
    nc.compile()
    _PROG[key] = nc
    return nc


# ------------------------------------------------------------------ kernel
def kernel(edge_attr, msg_emb, edge_dst, num_nodes, We, be, W1, b1, W4,
           **_unused):
    assert int(num_nodes) == N
    in_maps, slotmaps = pack_inputs(edge_attr, msg_emb, edge_dst,
                                    We, be, W1, b1, W4)
    nc = build_program()

    from concourse.bass_utils import run_bass_kernel_spmd
    trace = os.environ.get("GNN_TRACE", "0") == "1"
    res = run_bass_kernel_spmd(nc, in_maps, core_ids=list(range(NCORES)),
                               trace=trace)
    kernel.last_results = res
    return unpack_output(res.results, slotmaps)
